# revision 14
# baseline (speedup 1.0000x reference)
"""Trainium2 Bass kernel for nn_AdditiveAttention (additive attention, eval mode).

Math (faithful to the reference, including its use of q on both sides):
    A = q @ W1.T                      (bz, L, h)
    B = q @ W2.T + b2                 (bz, L, h)
    S[b,i,j] = sum_h w_h * tanh(A[b,i,h] + B[b,j,h])
    out = softmax_j(mask ? S : -1e9) @ v

tanh is expanded as sum_m c_m sin(lam_m x) with free (non-harmonic)
frequencies fit offline against the weighted-range tanh; sin of a sum
splits into sin/cos products, turning the score cube into TensorEngine
matmuls over the h contraction (2 matmuls per harmonic per j-block).

Per-core pipeline (data-parallel over batch, one batch per NeuronCore):
  1. A^T/B^T = W{1,2}T.T @ qT (fp16 inputs, fp32 PSUM accumulate)
  2. per m: FRAC2 fused DVE op -> phases; ACT Sin -> features;
     Pool (gpsimd) tensor_scalar -> c_m*w_h weighted A-features;
     PE matmuls accumulate S^T in PSUM.
  3. expS^T = Exp(S^T + maskbias) per j-block, interleaved with the
     out/rowsum matmul accumulation over j-blocks (pass 1: ib 0,1).
  4. fast reciprocal of row sums; normalization folded into Pool
     tensor_scalar PSUM->SBUF copies; DMA out.

Engine budget per harmonic (sim): DVE 2.2us (FRAC2) | ACT 1.9us (Sin)
| Pool 1.6us (weights) | PE 1.8us (8 matmuls + dummy).

Schedule fixes vs the 82us baseline: input DMAs split across SP/ACT/
SWDGE queues; PE p-state warm-up matmuls at t=0 (clock ramps 0.65 ->
2.4GHz only after ~3us of continuous busy); activation-table load
hoisted to t=0 via a dummy Sin; M reduced 7 -> 5 by refitting the
frequencies; weight-muls moved to the otherwise-idle Pool engine;
exp/out-matmul tail software-pipelined per j-block.

HW-quirk notes (from the previous session, kept):
  - walrus allows only ONE sync wait per instruction; a tiny dummy PE
    matmul per harmonic absorbs the Pool-side wait so the f32r score
    matmuls carry <= 1 wait.
  - f32r matmul *weights* (stationary operand) must be produced by a
    compute engine; fp16 weights are also passed through a DVE copy.
  - matmuls with free dim 1 are invalid ISA; tiny matmuls use N=4.
  - Sin's spline domain is [-pi, pi]: scale 6.2831845 < 2*pi keeps
    0.5-turn phases inside the table range.
"""

from contextlib import ExitStack

import numpy as np

# ----------------------------------------------------------------------------
# sin-expansion fits of tanh (offline, fit to the weighted input range)
# ----------------------------------------------------------------------------
_NLFIT = {
    4: ([1.2423547666769494, 0.3325054528359243, 0.1776347981157262,
         0.06424781885281551],
        [0.252482991992673, 0.7622940291865303, 1.2901203565992851,
         2.326009095757841]),
    5: ([1.2428171087697242, 0.3428135515932597, 0.14004185415786305,
         0.07947223727081101, 0.028430924834517316],
        [0.25055915256498046, 0.7541615078321168, 1.2681925589543304,
         1.8036285973988113, 2.851352718405678]),
    6: ([1.2430984002714982, 0.3441014558631853, 0.14631080893778753,
         0.06538101504163517, 0.015703582893894306, 0.028756089335022002],
        [0.24698241100931093, 0.7438185697773547, 1.2481789499658151,
         1.7620085141002502, 2.828141262473772, 2.2894316026532087]),
}

M_TERMS = 5
MAGIC = 12582912.0            # 1.5 * 2**23: fp32 add rounds to nearest int
TWO_PI_SAFE = 6.2831845       # slightly below 2*pi: keep |arg| <= pi
L = 512
H = 100
D = 512
NCORES = 8

_cached = {}
_CUR_M = [M_TERMS]


def _register_frac_op():
    """Fused DVE op: out = u - round(u), u = in0*s0 + in1 (in1 = shift).

    round() via the fp32 magic-number trick: (u + 1.5*2^23) - 1.5*2^23.
    5 ALU stages on the 8-stage DVE pipeline.
    """
    import concourse.dve_ops as dve_ops
    from concourse.dve_spec import Spec, Src0, Src1, C0, C1, lower, _has_src1
    from concourse.dve_uop import DveOpSpec

    def _mkop(name, body, ref):
        if name in dve_ops._SUB_OPCODE_FOR_NAME:
            return [o for o in dve_ops.OPS if o.name == name][0]
        spec = Spec(body=body, reference=ref)
        row = max(dve_ops._SUB_OPCODE_FOR_NAME.values()) + 1
        assert row < 0x20
        dve_ops._SUB_OPCODE_FOR_NAME[name] = row
        shas = {}
        for ver in ("v3",):
            uops = lower(spec, ver=ver)
            s = DveOpSpec(name=name, opcode=row, uops=uops, rd1_en=_has_src1(spec))
            shas[ver] = s.sha(ver)
        op = dve_ops.DveOp(name, spec, subdim=False, uops_sha=shas)
        dve_ops.OPS.append(op)
        dve_ops.CUSTOM_DVE_SPECS[name] = spec
        return op

    _u2 = Src0 * C0 + Src1
    def _ref2(in0, in1, c0, c1, c2):
        u = (in0.astype(np.float32) * np.float32(c0) + in1.astype(np.float32)).astype(np.float32)
        k = ((u + np.float32(c1)).astype(np.float32) - np.float32(c1)).astype(np.float32)
        return (u - k).astype(np.float32)
    op2 = _mkop("FRAC2_CENTERED_AA50", _u2 - ((_u2 + C1) - C1), _ref2)
    return op2


def build_nc(m_terms=M_TERMS, repeat=0):
    import concourse.bass as bass
    import concourse.bacc as bacc
    import concourse.tile as tile
    import concourse.mybir as mybir

    FRAC2 = _register_frac_op()
    f32 = mybir.dt.float32
    f32r = mybir.dt.float32r
    f16 = mybir.dt.float16
    AF = mybir.ActivationFunctionType
    ALU = mybir.AluOpType
    C, LAM = _NLFIT[m_terms]
    _CUR_M[0] = m_terms

    nc = bacc.Bacc("TRN2", target_bir_lowering=False, debug=False)

    qT_d = nc.dram_tensor("qT16", (D, L), f16, kind="ExternalInput").ap()
    v_d = nc.dram_tensor("vin", (L, D), f32r, kind="ExternalInput").ap()
    wp_d = nc.dram_tensor("wpack16", (D, 256), f16, kind="ExternalInput").ap()
    aux_d = nc.dram_tensor("auxpack", (H, 32), f32, kind="ExternalInput").ap()
    maskb_d = nc.dram_tensor("maskb", (L, 1), f32, kind="ExternalInput").ap()
    out_d = nc.dram_tensor("outp", (L, D), f32, kind="ExternalOutput").ap()

    with tile.TileContext(nc) as tc, ExitStack() as ctx:
        if repeat:
            loop_cm = tc.For_i(0, repeat, 1,
                               hint_engines=(mybir.EngineType.PE,))
            loop_cm.__enter__()
        const = ctx.enter_context(tc.tile_pool(name="const", bufs=1))
        phases = ctx.enter_context(tc.tile_pool(name="phases", bufs=3))
        feats = ctx.enter_context(tc.tile_pool(name="feats", bufs=3))
        faws = ctx.enter_context(tc.tile_pool(name="faws", bufs=3))
        expp = ctx.enter_context(tc.tile_pool(name="expp", bufs=1))
        outp = ctx.enter_context(tc.tile_pool(name="outp", bufs=2))
        psum = ctx.enter_context(tc.tile_pool(name="psum", bufs=1, space="PSUM"))

        # ---- t=0: warm-up + early activation-table load ----------------
        # PE p-state ramps with continuous busy; keep PE running from t=0
        # so the A/B matmuls hit full clock.
        warm_f = const.tile([128, 512], f32, tag="warm_f")
        nc.vector.memset(warm_f, 1.0)
        warm_lhs = const.tile([128, 4], f32r, tag="wlhs")
        nc.vector.tensor_scalar(out=warm_lhs, in0=warm_f[:, 0:4],
                                scalar1=1.0, scalar2=None, op0=ALU.mult)
        warm_rhs = const.tile([128, 512], f32r, tag="wrhs")
        nc.vector.tensor_scalar(out=warm_rhs, in0=warm_f, scalar1=1.0,
                                scalar2=None, op0=ALU.mult)
        tinys = const.tile([128, 4], f32, tag="tinys")
        nc.vector.memset(tinys, 0.0)
        tsin = const.tile([128, 4], f32, tag="tsin")
        # triggers the trig table load immediately (Sin table holds
        # copy/identity too, so no further load until Exp)
        nc.scalar.activation(out=tsin, in_=tinys, func=AF.Sin,
                             scale=TWO_PI_SAFE)
        for w in range(3):
            pwarm = psum.tile([128, 512], f32, tag="po", bufs=2, name=f"pwarm{w}")
            nc.tensor.matmul(pwarm[0:4, :], warm_lhs, warm_rhs,
                             start=True, stop=True)

        # ---- input DMAs (split across queues) ---------------------------
        # SP queue: wpack first (gates the A/B matmuls via the DVE copy),
        # then qT chunks, auxpack, maskb
        wpraw = const.tile([128, 4, 256], f16, tag="wpraw")
        nc.sync.dma_start(out=wpraw,
                          in_=wp_d.rearrange("(c p) x -> p c x", p=128))
        qT = []
        for c in range(4):
            t = const.tile([128, L], f16, tag=f"qT{c}")
            nc.sync.dma_start(out=t, in_=qT_d[c * 128:(c + 1) * 128, :])
            qT.append(t)
        aux = const.tile([H, 32], f32, tag="aux")
        nc.sync.dma_start(out=aux, in_=aux_d[:, :])
        maskb = const.tile([128, 4], f32, tag="maskb")
        nc.sync.dma_start(out=maskb,
                          in_=maskb_d.rearrange("(c p) one -> p (c one)", p=128))
        cw = aux[:, 0:16]

        # DVE-copy of the weights (stationary operands must come from a
        # compute engine)
        wpk = const.tile([128, 4, 256], f16, tag="wpk")
        nc.vector.tensor_scalar(out=wpk, in0=wpraw, scalar1=1.0,
                                scalar2=None, op0=ALU.mult)
        w1t = [wpk[:, c, 0:128] for c in range(4)]
        w2t = [wpk[:, c, 128:256] for c in range(4)]

        # Pool SWDGE queue: v (needed only at the end)
        vwide = const.tile([128, 4, D], f32r, tag="vwide")
        nc.gpsimd.dma_start(
            out=vwide,
            in_=v_d.rearrange("(c p) d -> p c d", p=128))
        vsb = [vwide[:, c, :] for c in range(4)]

        shifts = const.tile([H, 2], f32, tag="shifts")
        nc.vector.memset(shifts[:, 0:1], 0.0)
        nc.vector.memset(shifts[:, 1:2], 0.25)
        ones = const.tile([128, 4], f32r, tag="ones")
        nc.vector.tensor_scalar(out=ones, in0=warm_f[:, 0:4],
                                scalar1=1.0, scalar2=None, op0=ALU.mult)

        # ---- A^T / B^T  (fp16 matmuls; A first so FRAC2 starts early) --
        # No SBUF copies: the per-m FRAC2 calls read ps_a/ps_b (PSUM)
        # directly; the b2 bias is folded into the B-side shift stream
        # (host precomputes b2*lam_m/2pi + {0, 0.25} per m).
        ps_a = psum.tile([128, L], f32, tag="ab", bufs=2, name="ps_a")
        ps_b = psum.tile([128, L], f32, tag="ab", bufs=2, name="ps_b")
        for c in range(4):
            nc.tensor.matmul(ps_a, w1t[c], qT[c],
                             start=(c == 0), stop=(c == 3))
        for c in range(4):
            nc.tensor.matmul(ps_b, w2t[c], qT[c],
                             start=(c == 0), stop=(c == 3))

        # rowsum accumulator; reuses ps_a's slot (free after the last
        # FRAC2 A-call)
        auxt = psum.tile([128, 64], f32, tag="ab", bufs=2, name="auxt")

        # ---- score matmul accumulators ---------------------------------
        st = [psum.tile([128, L], f32, tag="st", bufs=4, name=f"st{jb}")
              for jb in range(4)]

        # ---- per-harmonic feature generation + accumulation ------------
        # ph/ft layout: [sinA | cosA | sinB | cosB]; the A-half (cols
        # 0:2L) and B-half (2L:4L) are produced by separate FRAC2/Sin
        # calls so the pipeline advances in half-harmonic steps.
        for mi in range(m_terms):
            lam_over_2pi = float(np.float32(LAM[mi] / (2.0 * np.pi)))

            ph = phases.tile([H, 4 * L], f32, tag="ph")
            psa_h = ps_a[:H, :]
            psb_h = ps_b[:H, :]
            psa_rep = bass.AP(
                tensor=psa_h.tensor, offset=psa_h.offset,
                ap=[psa_h.ap[0], [0, 2], [1, L]])
            psb_rep = bass.AP(
                tensor=psb_h.tensor, offset=psb_h.offset,
                ap=[psb_h.ap[0], [0, 2], [1, L]])
            shift_rep = bass.AP(
                tensor=shifts.tensor, offset=shifts.offset,
                ap=[shifts.ap[0], [1, 2], [0, L]])
            bsh = aux[:, 16 + 2 * mi:16 + 2 * mi + 2]
            bshift_rep = bass.AP(
                tensor=bsh.tensor, offset=bsh.offset,
                ap=[bsh.ap[0], [1, 2], [0, L]])
            nc.vector._custom_dve(
                FRAC2, out=ph[:, 0:2 * L], in0=psa_rep, in1=shift_rep,
                s0=lam_over_2pi, s1=MAGIC, imm2=0.0)
            if mi == 0:
                fil1 = psum.tile([128, 128], f32, tag="po", bufs=2,
                                 name="fil1")
                nc.tensor.matmul(fil1[0:4, :], tinys[:100, :],
                                 ph[:, 0:128], start=True, stop=True)
            nc.vector._custom_dve(
                FRAC2, out=ph[:, 2 * L:4 * L], in0=psb_rep, in1=bshift_rep,
                s0=lam_over_2pi, s1=MAGIC, imm2=0.0)

            ft = feats.tile([H, 4 * L], f32r, tag="ft")
            nc.scalar.activation(out=ft[:, 0:2 * L], in_=ph[:, 0:2 * L],
                                 func=AF.Sin, scale=TWO_PI_SAFE)
            if mi == 0:
                fil2 = psum.tile([128, 128], f32, tag="po", bufs=2,
                                 name="fil2")
                nc.tensor.matmul(fil2[0:4, :], warm_lhs[:100, :],
                                 ft[:, 0:128], start=True, stop=True)
            nc.scalar.activation(out=ft[:, 2 * L:4 * L], in_=ph[:, 2 * L:4 * L],
                                 func=AF.Sin, scale=TWO_PI_SAFE)

            # weight the A-side features by c_m * w_h on the Pool engine:
            # faw = [cw*sinA | cw*cosA] (contiguous A-half)
            faw = faws.tile([H, 2 * L], f32r, tag="faw")
            nc.gpsimd.tensor_scalar(out=faw, in0=ft[:, 0:2 * L],
                                    scalar1=cw[:, mi:mi + 1],
                                    scalar2=None, op0=ALU.mult)

            # tiny PE matmul reading faw: absorbs the Pool-side wait so the
            # real (self-loading f32r) matmuls below carry <= 1 sync wait
            scr = psum.tile([128, 4], f32, tag="po", bufs=2, name=f"scr{mi}")
            nc.tensor.matmul(scr[:, 0:4], faw[:, 0:128], faw[:, 0:4],
                             start=True, stop=True)

            first = (mi == 0)
            last = (mi == m_terms - 1)
            for jb in range(4):
                # S^T[j,i] += cosB[:,j].T @ (cw sinA)  +  sinB[:,j].T @ (cw cosA)
                lhs_cosB = ft[:, 3 * L + jb * 128: 3 * L + (jb + 1) * 128]
                lhs_sinB = ft[:, 2 * L + jb * 128: 2 * L + (jb + 1) * 128]
                nc.tensor.matmul(st[jb], lhs_cosB,
                                 faw[:, 0:L],
                                 start=first, stop=False)
                nc.tensor.matmul(st[jb], lhs_sinB,
                                 faw[:, L:2 * L],
                                 start=False, stop=last)

        # ---- tail: exp + rowsum/out accumulation, pipelined per jb -----
        # po0/po1 live in the "po" slots; po2/po3 reuse the st slots that
        # free as the exps consume them (exact-fit 8 PSUM banks)
        est = []
        po = [psum.tile([128, D], f32, tag="po", bufs=2, name=f"po{ib}")
              for ib in range(2)]
        po += [psum.tile([128, D], f32, tag="st", bufs=4, name=f"po{ib}")
               for ib in (2, 3)]
        for jb in range(4):
            t = expp.tile([128, L], f32r, tag=f"est{jb}")
            nc.scalar.activation(out=t, in_=st[jb], func=AF.Exp,
                                 bias=maskb[:, jb:jb + 1], scale=1.0)
            est.append(t)
            # per-jb partial rowsums first (frees the reciprocal early),
            # 16 independent one-shot groups
            for ib in range(4):
                nc.tensor.matmul(auxt[:, ib * 16 + jb * 4:
                                      ib * 16 + jb * 4 + 4],
                                 t[:, ib * 128:(ib + 1) * 128],
                                 ones, start=True, stop=True)
            for ib in range(4):
                nc.tensor.matmul(po[ib], t[:, ib * 128:(ib + 1) * 128],
                                 vsb[jb], start=(jb == 0), stop=(jb == 3))

        # rowsum_i = sum of the 4 partials; fast reciprocal
        rsum = const.tile([128, 4], f32, tag="rsum")
        nc.vector.tensor_reduce(
            out=rsum, in_=bass.AP(
                tensor=auxt.tensor, offset=auxt.offset,
                ap=[auxt.ap[0], [16, 4], [4, 4]]),
            axis=mybir.AxisListType.X, op=ALU.add)
        rc = const.tile([128, 4], f32, tag="rc")
        nc.vector.reciprocal_approx_fast(out=rc, in_=rsum)

        # ---- out = po * recip, PSUM->SBUF copies on DVE + ACT ----------
        # (GPSIMD/Pool cannot read PSUM, per the BIR verifier)
        owide = outp.tile([128, 4, D], f32, tag="owide")
        for ib in range(4):
            if ib % 2 == 0:
                nc.vector.tensor_scalar(out=owide[:, ib, :], in0=po[ib],
                                        scalar1=rc[:, ib:ib + 1],
                                        scalar2=None, op0=ALU.mult)
            else:
                nc.scalar.mul(out=owide[:, ib, :], in_=po[ib],
                              mul=rc[:, ib:ib + 1])
            eng = nc.sync if ib < 2 else nc.scalar
            eng.dma_start(out=out_d[ib * 128:(ib + 1) * 128, :],
                          in_=owide[:, ib, :])

        if repeat:
            loop_cm.__exit__(None, None, None)

    nc.compile()
    return nc


def _get_nc(m_terms=M_TERMS, repeat=0):
    key = (m_terms, repeat)
    if key not in _cached:
        _cached[key] = build_nc(m_terms, repeat)
    return _cached[key]


def make_in_maps(q, v, mask, W1, W2, b2, w_out):
    q = np.asarray(q, dtype=np.float32)
    v = np.asarray(v, dtype=np.float32)
    mask = np.asarray(mask)
    W1 = np.asarray(W1, dtype=np.float32)
    W2 = np.asarray(W2, dtype=np.float32)
    b2 = np.asarray(b2, dtype=np.float32)
    w_out = np.asarray(w_out, dtype=np.float32)

    w1tp = np.zeros((D, 128), np.float16); w1tp[:, :H] = W1.T.astype(np.float16)
    w2tp = np.zeros((D, 128), np.float16); w2tp[:, :H] = W2.T.astype(np.float16)
    wpack = np.ascontiguousarray(np.concatenate([w1tp, w2tp], axis=1))
    C, LAM = _NLFIT[_CUR_M[0]]
    auxp = np.zeros((H, 32), np.float32)
    auxp[:, :len(C)] = np.asarray(C, np.float32)[None, :] * w_out[:, None]
    for m, lam in enumerate(LAM):
        s = np.float32(np.float32(lam / (2.0 * np.pi)))
        auxp[:, 16 + 2 * m] = b2.astype(np.float32) * s
        auxp[:, 16 + 2 * m + 1] = b2.astype(np.float32) * s + np.float32(0.25)
    in_maps = []
    for b in range(NCORES):
        maskb = ((mask[b].astype(np.float32) - 1.0) * 1e9).reshape(L, 1)
        in_maps.append({
            "qT16": np.ascontiguousarray(q[b].T.astype(np.float16)),
            "vin": np.ascontiguousarray(v[b]),
            "wpack16": wpack,
            "auxpack": auxp,
            "maskb": np.ascontiguousarray(maskb),
        })
    return in_maps


def run(q, k, v, mask, W1, W2, b2, w_out, trace=False, m_terms=M_TERMS):
    from concourse.bass_utils import run_bass_kernel_spmd

    nc = _get_nc(m_terms)
    in_maps = make_in_maps(q, v, mask, W1, W2, b2, w_out)
    res = run_bass_kernel_spmd(nc, in_maps, core_ids=list(range(NCORES)),
                               trace=trace)
    out = np.stack([res.results[b]["outp"] for b in range(NCORES)])
    return out.astype(np.float32), res


def kernel(q, k, v, mask, W1, W2, b2, w_out):
    out, _ = run(q, k, v, mask, W1, W2, b2, w_out, trace=False)
    return out


# revision 19
# speedup vs baseline: 2.6591x; 2.6591x over previous
"""Trainium2 Bass kernel for nn_AdditiveAttention (additive attention, eval mode).

Math (faithful to the reference, including its use of q on both sides):
    A = q @ W1.T                      (bz, L, h)
    B = q @ W2.T + b2                 (bz, L, h)
    S[b,i,j] = sum_h w_h * tanh(A[b,i,h] + B[b,j,h])
    out = softmax_j(mask ? S : -1e9) @ v

tanh is expanded as sum_m c_m sin(lam_m x) with free (non-harmonic)
frequencies fit offline against the weighted-range tanh; sin of a sum
splits into sin/cos products, turning the score cube into TensorEngine
matmuls over the h contraction (2 matmuls per harmonic per j-block).

Per-core pipeline (data-parallel over batch, one batch per NeuronCore):
  1. A^T/B^T = W{1,2}T.T @ qT (fp16 inputs, fp32 PSUM accumulate)
  2. per m: FRAC2 fused DVE op -> phases; ACT Sin -> features;
     Pool (gpsimd) tensor_scalar -> c_m*w_h weighted A-features;
     PE matmuls accumulate S^T in PSUM.
  3. expS^T = Exp(S^T + maskbias) per j-block, interleaved with the
     out/rowsum matmul accumulation over j-blocks (pass 1: ib 0,1).
  4. fast reciprocal of row sums; normalization folded into Pool
     tensor_scalar PSUM->SBUF copies; DMA out.

Engine budget per harmonic (sim): DVE 2.2us (FRAC2) | ACT 1.9us (Sin)
| Pool 1.6us (weights) | PE 1.8us (8 matmuls + dummy).

Schedule fixes vs the 82us baseline: input DMAs split across SP/ACT/
SWDGE queues; PE p-state warm-up matmuls at t=0 (clock ramps 0.65 ->
2.4GHz only after ~3us of continuous busy); activation-table load
hoisted to t=0 via a dummy Sin; M reduced 7 -> 5 by refitting the
frequencies; weight-muls moved to the otherwise-idle Pool engine;
exp/out-matmul tail software-pipelined per j-block.

HW-quirk notes (from the previous session, kept):
  - walrus allows only ONE sync wait per instruction; a tiny dummy PE
    matmul per harmonic absorbs the Pool-side wait so the f32r score
    matmuls carry <= 1 wait.
  - f32r matmul *weights* (stationary operand) must be produced by a
    compute engine; fp16 weights are also passed through a DVE copy.
  - matmuls with free dim 1 are invalid ISA; tiny matmuls use N=4.
  - Sin's spline domain is [-pi, pi]: scale 6.2831845 < 2*pi keeps
    0.5-turn phases inside the table range.
"""

from contextlib import ExitStack

import numpy as np

# ----------------------------------------------------------------------------
# sin-expansion fits of tanh (offline, fit to the weighted input range)
# ----------------------------------------------------------------------------
_NLFIT = {
    4: ([1.2233079121168684, 0.3315231604395483, 0.13526990270924266,
         0.04129792667131724],
        [0.29144632103685597, 0.8779629607024633, 1.5758932581313203,
         2.462887876842787]),
    5: ([1.2704415998180179, 0.34086419711332094, 0.13219952401172733,
         0.07862601278273276, 0.028771668076239716],
        [0.24942747621995154, 0.7865750481711155, 1.2778216329413166,
         1.8679282795341892, 2.778644388459502]),
    6: ([1.2430984002714982, 0.3441014558631853, 0.14631080893778753,
         0.06538101504163517, 0.015703582893894306, 0.028756089335022002],
        [0.24698241100931093, 0.7438185697773547, 1.2481789499658151,
         1.7620085141002502, 2.828141262473772, 2.2894316026532087]),
}

M_TERMS = 4
MAGIC = 12582912.0            # 1.5 * 2**23: fp32 add rounds to nearest int
TWO_PI_SAFE = 6.2831845       # slightly below 2*pi: keep |arg| <= pi
L = 512
H = 100
D = 512
NCORES = 8

_cached = {}
_CUR_M = [M_TERMS]


def _register_frac_op():
    """Fused DVE op: out = u - round(u), u = in0*s0 + in1 (in1 = shift).

    round() via the fp32 magic-number trick: (u + 1.5*2^23) - 1.5*2^23.
    5 ALU stages on the 8-stage DVE pipeline.
    """
    import concourse.dve_ops as dve_ops
    from concourse.dve_spec import Spec, Src0, Src1, C0, C1, lower, _has_src1
    from concourse.dve_uop import DveOpSpec

    def _mkop(name, body, ref):
        if name in dve_ops._SUB_OPCODE_FOR_NAME:
            return [o for o in dve_ops.OPS if o.name == name][0]
        spec = Spec(body=body, reference=ref)
        row = max(dve_ops._SUB_OPCODE_FOR_NAME.values()) + 1
        assert row < 0x20
        dve_ops._SUB_OPCODE_FOR_NAME[name] = row
        shas = {}
        for ver in ("v3",):
            uops = lower(spec, ver=ver)
            s = DveOpSpec(name=name, opcode=row, uops=uops, rd1_en=_has_src1(spec))
            shas[ver] = s.sha(ver)
        op = dve_ops.DveOp(name, spec, subdim=False, uops_sha=shas)
        dve_ops.OPS.append(op)
        dve_ops.CUSTOM_DVE_SPECS[name] = spec
        return op

    _u2 = Src0 * C0 + Src1
    def _ref2(in0, in1, c0, c1, c2):
        u = (in0.astype(np.float32) * np.float32(c0) + in1.astype(np.float32)).astype(np.float32)
        k = ((u + np.float32(c1)).astype(np.float32) - np.float32(c1)).astype(np.float32)
        return (u - k).astype(np.float32)
    op2 = _mkop("FRAC2_CENTERED_AA50", _u2 - ((_u2 + C1) - C1), _ref2)
    return op2


def build_nc(m_terms=M_TERMS, repeat=0, pool_faw=False, swdge_v=False,
             merged=True):
    import concourse.bass as bass
    import concourse.bacc as bacc
    import concourse.tile as tile
    import concourse.mybir as mybir

    FRAC2 = _register_frac_op()
    f32 = mybir.dt.float32
    f32r = mybir.dt.float32r
    f16 = mybir.dt.float16
    AF = mybir.ActivationFunctionType
    ALU = mybir.AluOpType
    C, LAM = _NLFIT[m_terms]
    _CUR_M[0] = m_terms

    nc = bacc.Bacc("TRN2", target_bir_lowering=False, debug=False)

    qT_d = nc.dram_tensor("qT16", (D, L), f16, kind="ExternalInput").ap()
    v_d = nc.dram_tensor("vin", (L, D), f32r, kind="ExternalInput").ap()
    wp_d = nc.dram_tensor("wpack16", (D, 256), f16, kind="ExternalInput").ap()
    aux_d = nc.dram_tensor("auxpack", (H, 48), f32, kind="ExternalInput").ap()
    maskb_d = nc.dram_tensor("maskb", (L, 1), f32, kind="ExternalInput").ap()
    out_d = nc.dram_tensor("outp", (L, D), f32, kind="ExternalOutput").ap()

    with tile.TileContext(nc) as tc, ExitStack() as ctx:
        if repeat:
            loop_cm = tc.For_i(0, repeat, 1,
                               hint_engines=(mybir.EngineType.PE,))
            loop_cm.__enter__()
        const = ctx.enter_context(tc.tile_pool(name="const", bufs=1))
        phases = ctx.enter_context(tc.tile_pool(name="phases", bufs=3))
        feats = ctx.enter_context(tc.tile_pool(name="feats", bufs=3))
        faws = ctx.enter_context(tc.tile_pool(name="faws", bufs=3))
        expp = ctx.enter_context(tc.tile_pool(name="expp", bufs=1))
        outp = ctx.enter_context(tc.tile_pool(name="outp", bufs=2))
        psum = ctx.enter_context(tc.tile_pool(name="psum", bufs=1, space="PSUM"))

        # ---- t=0: warm-up + early activation-table load ----------------
        # PE p-state ramps with continuous busy; keep PE running from t=0
        # so the A/B matmuls hit full clock.
        warm_f = const.tile([128, 512], f32, tag="warm_f")
        nc.vector.memset(warm_f, 1.0)
        warm_lhs = const.tile([128, 4], f32r, tag="wlhs")
        nc.vector.tensor_scalar(out=warm_lhs, in0=warm_f[:, 0:4],
                                scalar1=1.0, scalar2=None, op0=ALU.mult)
        warm_rhs = const.tile([128, 512], f32r, tag="wrhs")
        nc.vector.tensor_scalar(out=warm_rhs, in0=warm_f, scalar1=1.0,
                                scalar2=None, op0=ALU.mult)
        tinys = const.tile([128, 4], f32, tag="tinys")
        nc.vector.memset(tinys, 0.0)
        tsin = const.tile([128, 4], f32, tag="tsin")
        # triggers the trig table load immediately (Sin table holds
        # copy/identity too, so no further load until Exp)
        nc.scalar.activation(out=tsin, in_=tinys, func=AF.Sin,
                             scale=TWO_PI_SAFE)
        for w in range(3):
            pwarm = psum.tile([128, 512], f32, tag="po", bufs=2, name=f"pwarm{w}")
            nc.tensor.matmul(pwarm[0:4, :], warm_lhs, warm_rhs,
                             start=True, stop=True)

        # ---- input DMAs (split across queues) ---------------------------
        # SP queue: wpack first (gates the A/B matmuls via the DVE copy),
        # then qT chunks, auxpack, maskb
        wpraw = const.tile([128, 4, 256], f16, tag="wpraw")
        nc.sync.dma_start(out=wpraw,
                          in_=wp_d.rearrange("(c p) x -> p c x", p=128))
        qT = []
        for c in range(4):
            t = const.tile([128, L], f16, tag=f"qT{c}")
            nc.sync.dma_start(out=t, in_=qT_d[c * 128:(c + 1) * 128, :])
            qT.append(t)
        aux = const.tile([H, 48], f32, tag="aux")
        nc.sync.dma_start(out=aux, in_=aux_d[:, :])
        maskb = const.tile([128, 4], f32, tag="maskb")
        nc.sync.dma_start(out=maskb,
                          in_=maskb_d.rearrange("(c p) one -> p (c one)", p=128))
        cw = aux[:, 0:16]

        # DVE-copy of the weights (stationary operands must come from a
        # compute engine)
        wpk = const.tile([128, 4, 256], f16, tag="wpk")
        nc.vector.tensor_scalar(out=wpk, in0=wpraw, scalar1=1.0,
                                scalar2=None, op0=ALU.mult)
        w1t = [wpk[:, c, 0:128] for c in range(4)]
        w2t = [wpk[:, c, 128:256] for c in range(4)]

        # v (needed only at the end): Pool SWDGE queue or late on SP
        vwide = const.tile([128, 4, D], f32r, tag="vwide")
        veng = nc.gpsimd if swdge_v else nc.sync
        veng.dma_start(
            out=vwide,
            in_=v_d.rearrange("(c p) d -> p c d", p=128))
        vsb = [vwide[:, c, :] for c in range(4)]

        shifts = const.tile([H, 2], f32, tag="shifts")
        nc.vector.memset(shifts[:, 0:1], 0.0)
        nc.vector.memset(shifts[:, 1:2], 0.25)
        ones = const.tile([128, 4], f32r, tag="ones")
        nc.vector.tensor_scalar(out=ones, in0=warm_f[:, 0:4],
                                scalar1=1.0, scalar2=None, op0=ALU.mult)

        # ---- A^T / B^T  (fp16 matmuls; A first so FRAC2 starts early) --
        # No SBUF copies: the per-m FRAC2 calls read ps_a/ps_b (PSUM)
        # directly; the b2 bias is folded into the B-side shift stream
        # (host precomputes b2*lam_m/2pi + {0, 0.25} per m).
        ps_a = psum.tile([128, L], f32, tag="ab", bufs=2, name="ps_a")
        ps_b = psum.tile([128, L], f32, tag="ab", bufs=2, name="ps_b")
        for c in range(4):
            nc.tensor.matmul(ps_a, w1t[c], qT[c],
                             start=(c == 0), stop=(c == 3))
        for c in range(4):
            nc.tensor.matmul(ps_b, w2t[c], qT[c],
                             start=(c == 0), stop=(c == 3))

        if merged:
            # single-call per-m path: FRAC2 reads a contiguous SBUF
            # [A^T | B^T]; plain copies (b2 is folded into the shifts)
            ATBT = const.tile([H, 2 * L], f32, tag="atbt")
            nc.vector.tensor_scalar(out=ATBT[:, 0:L], in0=ps_a[:H, :],
                                    scalar1=1.0, scalar2=None, op0=ALU.mult)
            nc.scalar.activation(out=ATBT[:, L:2 * L], in_=ps_b[:H, :],
                                 func=AF.Identity, bias=aux[:, 40:41],
                                 scale=1.0)

        # rowsum accumulator; reuses ps_a's slot (free after the last
        # FRAC2 A-call / the ATBT copies)
        auxt = psum.tile([128, 64], f32, tag="ab", bufs=2, name="auxt")

        # ---- score matmul accumulators ---------------------------------
        st = [psum.tile([128, L], f32, tag="st", bufs=4, name=f"st{jb}")
              for jb in range(4)]

        # ---- per-harmonic feature generation + accumulation ------------
        # ph/ft layout: [sinA | cosA | sinB | cosB]; the A-half (cols
        # 0:2L) and B-half (2L:4L) are produced by separate FRAC2/Sin
        # calls so the pipeline advances in half-harmonic steps.
        for mi in range(m_terms):
            lam_over_2pi = float(np.float32(LAM[mi] / (2.0 * np.pi)))

            ph = phases.tile([H, 4 * L], f32, tag="ph")
            if merged:
                # ph = [sinA | sinB | cosA | cosB] in one fused DVE pass;
                # in1 = [0, b2*lam/2pi, 0.25, b2*lam/2pi + 0.25] per L-block
                atbt_rep = bass.AP(
                    tensor=ATBT.tensor, offset=ATBT.offset,
                    ap=[ATBT.ap[0], [0, 2], [1, 2 * L]])
                shift_rep2 = bass.AP(
                    tensor=shifts.tensor, offset=shifts.offset,
                    ap=[shifts.ap[0], [1, 2], [0, 2 * L]])
                nc.vector._custom_dve(
                    FRAC2, out=ph, in0=atbt_rep, in1=shift_rep2,
                    s0=lam_over_2pi, s1=MAGIC, imm2=0.0)
            else:
                psa_h = ps_a[:H, :]
                psb_h = ps_b[:H, :]
                psa_rep = bass.AP(
                    tensor=psa_h.tensor, offset=psa_h.offset,
                    ap=[psa_h.ap[0], [0, 2], [1, L]])
                psb_rep = bass.AP(
                    tensor=psb_h.tensor, offset=psb_h.offset,
                    ap=[psb_h.ap[0], [0, 2], [1, L]])
                shift_rep = bass.AP(
                    tensor=shifts.tensor, offset=shifts.offset,
                    ap=[shifts.ap[0], [1, 2], [0, L]])
                bsh = aux[:, 17 + 4 * mi:17 + 4 * mi + 3]
                bshift_rep = bass.AP(
                    tensor=bsh.tensor, offset=bsh.offset,
                    ap=[bsh.ap[0], [2, 2], [0, L]])
                nc.vector._custom_dve(
                    FRAC2, out=ph[:, 0:2 * L], in0=psa_rep, in1=shift_rep,
                    s0=lam_over_2pi, s1=MAGIC, imm2=0.0)
            if mi == 0:
                fil1 = psum.tile([128, 128], f32, tag="po", bufs=2,
                                 name="fil1")
                nc.tensor.matmul(fil1[0:4, :], tinys[:100, :],
                                 ph[:, 0:128], start=True, stop=True)
            if not merged:
                nc.vector._custom_dve(
                    FRAC2, out=ph[:, 2 * L:4 * L], in0=psb_rep,
                    in1=bshift_rep, s0=lam_over_2pi, s1=MAGIC, imm2=0.0)

            ft = feats.tile([H, 4 * L], f32r, tag="ft")
            if merged:
                nc.scalar.activation(out=ft, in_=ph, func=AF.Sin,
                                     scale=TWO_PI_SAFE)
            else:
                nc.scalar.activation(out=ft[:, 0:2 * L], in_=ph[:, 0:2 * L],
                                     func=AF.Sin, scale=TWO_PI_SAFE)
            if mi == 0:
                fil2 = psum.tile([128, 128], f32, tag="po", bufs=2,
                                 name="fil2")
                nc.tensor.matmul(fil2[0:4, :], warm_lhs[:100, :],
                                 ft[:, 0:128], start=True, stop=True)
            if not merged:
                nc.scalar.activation(out=ft[:, 2 * L:4 * L],
                                     in_=ph[:, 2 * L:4 * L],
                                     func=AF.Sin, scale=TWO_PI_SAFE)

            # weight the A-side features by c_m * w_h:
            # faw = [cw*sinA | cw*cosA]; sin half on DVE, cos half on ACT
            # merged layout: sinA at 0:L, cosA at 2L:3L
            # split layout:  sinA at 0:L, cosA at L:2L
            cosA = ft[:, 2 * L:3 * L] if merged else ft[:, L:2 * L]
            # merged layout [sinA|sinB|cosA|cosB]: cosA at 2L, sinB at L
            faw = faws.tile([H, 2 * L], f32r, tag="faw")
            nc.vector.tensor_scalar(out=faw[:, 0:L], in0=ft[:, 0:L],
                                    scalar1=cw[:, mi:mi + 1],
                                    scalar2=None, op0=ALU.mult)
            nc.scalar.mul(out=faw[:, L:2 * L], in_=cosA,
                          mul=cw[:, mi:mi + 1])

            # tiny PE matmul reading faw: absorbs the Pool-side wait so the
            # real (self-loading f32r) matmuls below carry <= 1 sync wait
            scr = psum.tile([128, 4], f32, tag="po", bufs=2, name=f"scr{mi}")
            nc.tensor.matmul(scr[:, 0:4], faw[:, 0:128], faw[:, 0:4],
                             start=True, stop=True)

            first = (mi == 0)
            last = (mi == m_terms - 1)
            sinB_base = L if merged else 2 * L  # (split: [sinA|cosA|sinB|cosB])
            for jb in range(4):
                # S^T[j,i] += cosB[:,j].T @ (cw sinA)  +  sinB[:,j].T @ (cw cosA)
                lhs_cosB = ft[:, 3 * L + jb * 128: 3 * L + (jb + 1) * 128]
                lhs_sinB = ft[:, sinB_base + jb * 128:
                              sinB_base + (jb + 1) * 128]
                nc.tensor.matmul(st[jb], lhs_cosB,
                                 faw[:, 0:L],
                                 start=first, stop=False)
                nc.tensor.matmul(st[jb], lhs_sinB,
                                 faw[:, L:2 * L],
                                 start=False, stop=last)

        # ---- tail: exp + rowsum/out accumulation, pipelined per jb -----
        # po0/po1 live in the "po" slots; po2/po3 reuse the st slots that
        # free as the exps consume them (exact-fit 8 PSUM banks)
        est = []
        po = [psum.tile([128, D], f32, tag="po", bufs=2, name=f"po{ib}")
              for ib in range(2)]
        po += [psum.tile([128, D], f32, tag="st", bufs=4, name=f"po{ib}")
               for ib in (2, 3)]
        for jb in range(4):
            t = expp.tile([128, L], f32r, tag=f"est{jb}")
            nc.scalar.activation(out=t, in_=st[jb], func=AF.Exp,
                                 bias=maskb[:, jb:jb + 1], scale=1.0)
            est.append(t)
            # per-jb partial rowsums first (frees the reciprocal early),
            # 16 independent one-shot groups
            for ib in range(4):
                nc.tensor.matmul(auxt[:, ib * 16 + jb * 4:
                                      ib * 16 + jb * 4 + 4],
                                 t[:, ib * 128:(ib + 1) * 128],
                                 ones, start=True, stop=True)
            for ib in range(4):
                nc.tensor.matmul(po[ib], t[:, ib * 128:(ib + 1) * 128],
                                 vsb[jb], start=(jb == 0), stop=(jb == 3))

        # rowsum_i = sum of the 4 partials; fast reciprocal
        rsum = const.tile([128, 4], f32, tag="rsum")
        nc.vector.tensor_reduce(
            out=rsum, in_=bass.AP(
                tensor=auxt.tensor, offset=auxt.offset,
                ap=[auxt.ap[0], [16, 4], [4, 4]]),
            axis=mybir.AxisListType.X, op=ALU.add)
        rc = const.tile([128, 4], f32, tag="rc")
        nc.vector.reciprocal_approx_fast(out=rc, in_=rsum)

        # ---- out = po * recip, PSUM->SBUF copies on DVE + ACT ----------
        # (GPSIMD/Pool cannot read PSUM, per the BIR verifier)
        owide = outp.tile([128, 4, D], f32, tag="owide")
        for ib in range(4):
            if ib % 2 == 0:
                nc.vector.tensor_scalar(out=owide[:, ib, :], in0=po[ib],
                                        scalar1=rc[:, ib:ib + 1],
                                        scalar2=None, op0=ALU.mult)
            else:
                nc.scalar.mul(out=owide[:, ib, :], in_=po[ib],
                              mul=rc[:, ib:ib + 1])
            eng = nc.sync if ib < 2 else nc.scalar
            eng.dma_start(out=out_d[ib * 128:(ib + 1) * 128, :],
                          in_=owide[:, ib, :])

        if repeat:
            loop_cm.__exit__(None, None, None)

    nc.compile()
    return nc


def _get_nc(m_terms=M_TERMS, repeat=0, pool_faw=False, swdge_v=False,
            merged=True):
    key = (m_terms, repeat, pool_faw, swdge_v, merged)
    if key not in _cached:
        _cached[key] = build_nc(m_terms, repeat, pool_faw, swdge_v, merged)
    return _cached[key]


def make_in_maps(q, v, mask, W1, W2, b2, w_out):
    q = np.asarray(q, dtype=np.float32)
    v = np.asarray(v, dtype=np.float32)
    mask = np.asarray(mask)
    W1 = np.asarray(W1, dtype=np.float32)
    W2 = np.asarray(W2, dtype=np.float32)
    b2 = np.asarray(b2, dtype=np.float32)
    w_out = np.asarray(w_out, dtype=np.float32)

    w1tp = np.zeros((D, 128), np.float16); w1tp[:, :H] = W1.T.astype(np.float16)
    w2tp = np.zeros((D, 128), np.float16); w2tp[:, :H] = W2.T.astype(np.float16)
    wpack = np.ascontiguousarray(np.concatenate([w1tp, w2tp], axis=1))
    C, LAM = _NLFIT[_CUR_M[0]]
    auxp = np.zeros((H, 48), np.float32)
    auxp[:, :len(C)] = np.asarray(C, np.float32)[None, :] * w_out[:, None]
    for m, lam in enumerate(LAM):
        s = np.float32(np.float32(lam / (2.0 * np.pi)))
        base = 16 + 4 * m
        auxp[:, base + 0] = 0.0
        auxp[:, base + 1] = b2.astype(np.float32) * s
        auxp[:, base + 2] = 0.25
        auxp[:, base + 3] = b2.astype(np.float32) * s + np.float32(0.25)
    auxp[:, 40] = b2.astype(np.float32)
    in_maps = []
    for b in range(NCORES):
        maskb = ((mask[b].astype(np.float32) - 1.0) * 1e9).reshape(L, 1)
        in_maps.append({
            "qT16": np.ascontiguousarray(q[b].T.astype(np.float16)),
            "vin": np.ascontiguousarray(v[b]),
            "wpack16": wpack,
            "auxpack": auxp,
            "maskb": np.ascontiguousarray(maskb),
        })
    return in_maps


def run(q, k, v, mask, W1, W2, b2, w_out, trace=False, m_terms=M_TERMS):
    from concourse.bass_utils import run_bass_kernel_spmd

    nc = _get_nc(m_terms)
    in_maps = make_in_maps(q, v, mask, W1, W2, b2, w_out)
    res = run_bass_kernel_spmd(nc, in_maps, core_ids=list(range(NCORES)),
                               trace=trace)
    out = np.stack([res.results[b]["outp"] for b in range(NCORES)])
    return out.astype(np.float32), res


def kernel(q, k, v, mask, W1, W2, b2, w_out):
    out, _ = run(q, k, v, mask, W1, W2, b2, w_out, trace=False)
    return out
